# revision 4
# baseline (speedup 1.0000x reference)
"""MetaConv1d Trainium2 kernel — v13 (G scheme; bank-safe; balanced evacs).

Math (per sample n; device does the two big contractions in bf16/fp32-psum):
  W1c[d, c] = sum_m meta_aug[m, n] * w1aug[m, (d, c)]   (step1, tile-batched;
                                                         w1 bias via meta ones)
  G[e, t']  = sum_c W1c[e, c] * x[c, t']                (matmul1, e in 0..32)
  out[t, o] = sum_{e, j} G[e, t+j] * w2[e, (j, o)]      (matmul2, 3 taps)
Host adds the two cheap bias terms (bl linear; w2-bias x channel-sum conv).

Hardware rule discovered by probing: all matmuls targeting the same PSUM
bank must share the same operand partition base (mixing bases crashes the
device: NRT_EXEC_UNIT error). Hence:
  - G psum tiles are split by sample parity (even samples read x/W1c at
    partition base 0, odd at base 64): each (96, 256) tile takes 6
    same-parity samples at 3 output bases x 2 column halves.
  - out psum tiles are split by G partition base: each (126, 256) tile
    takes 4 samples whose G rows live at the same base 32g.

Cost-model-driven layout (see v4): per-sample matmuls keep N small; psum
evacuations batch 4-6 samples at full partition width; one x DMA and one
out DMA per 120-sample tile; x host-pretransposed to (c, t) bf16.

Sharding: batch*node dim (6624) split evenly over 8 cores (828 each).
"""

import numpy as np
import ml_dtypes

import concourse.mybir as mybir
import concourse.bacc as bacc
from concourse.tile import TileContext
from concourse.bass_utils import run_bass_kernel_spmd

BF16 = mybir.dt.bfloat16
F32 = mybir.dt.float32

B = 32
N = 207
BN = B * N            # 6624
L = 128
C = 64                # in channels
O = 64                # out channels
KK = 3
META = 32
MA = META + 1         # aug (ones row feeds w1 bias in step1)
LOUT = L - KK + 1     # 126
NCORES = 8
PER = BN // NCORES    # 828
NTS = 120             # samples per hypernet tile (multiple of 12)
DZ = 24               # samples per group (2 parities x 12), 12 at the tail
DC = 128              # w1augP inner block: (dup u=2) x (c=64)
TB = 4                # d-blocks per transpose-evac batch


def build_program(per=PER):
    """Per-core Bass program (identical on all 8 cores)."""
    assert per % 12 == 0
    nc = bacc.Bacc("TRN2", target_bir_lowering=False)

    # x image: partition p = (n%2)*64 + c ; col = (n//2)*L + t
    x_d = nc.dram_tensor("x", (128, (per // 2) * L), BF16, kind="ExternalInput")
    metaT_d = nc.dram_tensor("metaT", (MA, per), BF16, kind="ExternalInput")
    # w1augP: [m, d*64 + c] = W1[(c,d), m] (+ w1_b row); the base-64 copy of
    # W1cT is made by a second partition-shifted evac of each transpose
    w1augP_d = nc.dram_tensor("w1augP", (MA, META * C), BF16, kind="ExternalInput")
    # w2quad: rows 32i:32i+32 (i<3) hold w2P[e, j*64+o] (e<32, no bias row)
    w2quad_d = nc.dram_tensor("w2quad", (96, KK * O), BF16, kind="ExternalInput")
    identB_d = nc.dram_tensor("identB", (128, 128), BF16, kind="ExternalInput")
    # out image: [t, n*64 + o] (bf16; host upcasts + adds biases)
    out_d = nc.dram_tensor("out", (LOUT, per * O), BF16, kind="ExternalOutput")

    n_tiles = [(t, min(NTS, per - t)) for t in range(0, per, NTS)]

    with TileContext(nc) as tc:
        with (
            tc.tile_pool(name="const", bufs=1) as cpool,
            tc.tile_pool(name="wpool", bufs=2) as wpool,
            tc.tile_pool(name="xpool", bufs=2) as xpool,
            tc.tile_pool(name="gpool", bufs=3) as gpool,
            tc.tile_pool(name="opool", bufs=2) as opool,
            tc.tile_pool(name="pst", bufs=4, space="PSUM") as pst,
            tc.tile_pool(name="psg", bufs=2, space="PSUM") as psg,
            tc.tile_pool(name="pso", bufs=2, space="PSUM") as pso,
        ):
            w1augP = cpool.tile([MA, META * C], BF16)
            nc.sync.dma_start(w1augP[:, :], w1augP_d[:, :])
            w2quad = cpool.tile([96, KK * O], BF16)
            nc.sync.dma_start(w2quad[:, :], w2quad_d[:, :])
            identB = cpool.tile([128, 128], BF16)
            nc.sync.dma_start(identB[:, :], identB_d[:, :])

            for n0, nt in n_tiles:
                metaT_sb = wpool.tile([MA, nt], BF16, tag="metaT", padded_shape=[MA, NTS])
                nc.gpsimd.dma_start(metaT_sb[:, :], metaT_d[:, n0 : n0 + nt])

                # whole-tile x load: one DMA per 120 samples
                x_sb = xpool.tile(
                    [128, (nt // 2) * L], BF16, tag="xsb",
                    padded_shape=[128, (NTS // 2) * L],
                )
                nc.gpsimd.dma_start(
                    x_sb[:, :], x_d[:, n0 // 2 * L : (n0 + nt) // 2 * L]
                )

                # step1: W1out[n, (d, u, c)] batched over the tile
                W1out = wpool.tile(
                    [nt, META * C], BF16, tag="w1out", padded_shape=[NTS, META * C]
                )
                for k in range(META * C // 512):
                    ps1 = pst.tile([nt, 512], F32, tag="pstile", padded_shape=[NTS, 512])
                    nc.tensor.matmul(
                        ps1[:, :],
                        metaT_sb[:, :],
                        w1augP[:, k * 512 : (k + 1) * 512],
                        start=True,
                        stop=True,
                    )
                    if k % 2:
                        nc.vector.tensor_copy(W1out[:, k * 512 : (k + 1) * 512], ps1[:, :])
                    else:
                        nc.scalar.copy(W1out[:, k * 512 : (k + 1) * 512], ps1[:, :])

                # transpose d-blocks (nt, 128) -> (128, nt), TB per psum tile;
                # u-dup gives both 64-partition halves the same c-major rows.
                W1cT = wpool.tile([128, META * NTS], BF16, tag="w1ct")
                for d0 in range(0, META, TB):
                    psT = pst.tile(
                        [64, TB * nt], BF16, tag="pstile",
                        padded_shape=[64, TB * NTS],
                    )
                    for d in range(d0, d0 + TB):
                        nc.tensor.transpose(
                            psT[:, (d - d0) * nt : (d - d0 + 1) * nt],
                            W1out[:, d * C : (d + 1) * C],
                            identB[0:nt, 0:nt],
                        )
                    dst = (
                        W1cT[:, :]
                        .rearrange("p (e n) -> p e n", e=META)[
                            :, d0 : d0 + TB, 0:nt
                        ]
                    )
                    src = psT[:, :].rearrange("p (e n) -> p e n", n=nt)
                    eng0 = nc.vector if (d0 // TB) % 2 == 0 else nc.scalar
                    eng1 = nc.scalar if (d0 // TB) % 2 == 0 else nc.vector
                    if eng0 is nc.vector:
                        nc.vector.tensor_copy(dst[0:64], src)
                        nc.scalar.copy(dst[64:128], src)
                    else:
                        nc.scalar.copy(dst[0:64], src)
                        nc.vector.tensor_copy(dst[64:128], src)
                # lhsT view: [(n%2)*64 + c, e*NTS + ln] with e in 0..32
                W1cT_r = W1cT[:, :].rearrange("p (e n) -> p n e", e=META)

                # per-sample stage in dozens; G tiles split by parity so each
                # psum bank sees a single operand base (HW requirement).
                out_sb = opool.tile(
                    [LOUT, nt * O], BF16, tag="osb", padded_shape=[LOUT, NTS * O]
                )
                dma_from = 0
                out_r = out_sb[:, :].rearrange("t (n z) -> t n z", z=2 * O)
                for q0 in range(0, nt, DZ):
                    gw = min(DZ, nt - q0)      # 24, or 12 at the tail
                    hp = gw // 2               # samples per parity (12 or 6)
                    nb = hp // 3               # G column blocks (4 or 2)
                    Gs = []
                    for par in (0, 1):
                        psG = psg.tile(
                            [96, nb * L], F32, tag="psG", padded_shape=[96, 4 * L]
                        )
                        for i in range(hp):
                            ln = q0 + 2 * i + par
                            nc.tensor.matmul(
                                psG[(i % 3) * 32 : (i % 3) * 32 + 32,
                                    (i // 3) * L : (i // 3) * L + L],
                                W1cT_r[par * 64 : par * 64 + C, ln, :],
                                x_sb[par * 64 : par * 64 + C,
                                     (ln // 2) * L : (ln // 2) * L + L],
                                start=True,
                                stop=True,
                            )
                        Gsb = gpool.tile(
                            [96, nb * L], BF16, tag="gsb", padded_shape=[96, 4 * L]
                        )
                        nc.vector.tensor_copy(Gsb[:, :], psG[:, :])
                        Gs.append(Gsb)

                    # out groups by G partition base 32g: gw/3 samples each,
                    # slot order [e(g), o(g), e(g+3), o(g+3), ...] makes the
                    # out_sb destination a regular strided pattern.
                    for g in range(3):
                        ns_ = gw // 3          # 8 or 4 samples
                        psO = pso.tile(
                            [LOUT, ns_ * O], F32, tag="psO",
                            padded_shape=[LOUT, 8 * O],
                        )
                        k = 0
                        for i in range(g, hp, 3):
                            for par in (0, 1):
                                gcol = (i // 3) * L
                                for j in range(KK):
                                    nc.tensor.matmul(
                                        psO[:, k * O : (k + 1) * O],
                                        Gs[par][32 * g : 32 * g + 32,
                                                gcol + j : gcol + j + LOUT],
                                        w2quad[32 * g : 32 * g + 32,
                                               j * O : (j + 1) * O],
                                        start=(j == 0),
                                        stop=(j == KK - 1),
                                    )
                                k += 1
                        # sample pairs p2 = q0//2 + i for i in {g, g+3, ...}
                        p2 = q0 // 2 + g
                        nc.scalar.copy(
                            out_r[:, p2 : p2 + 3 * (ns_ // 2) - 2 : 3, :],
                            psO[:, :].rearrange("t (a z) -> t a z", z=2 * O),
                        )
                    qe = min(q0 + DZ, nt)
                    if (dma_from == 0 and 2 * qe >= nt) or qe == nt:
                        nc.gpsimd.dma_start(
                            out_d[:, (n0 + dma_from) * O : (n0 + qe) * O],
                            out_sb[:, dma_from * O : qe * O],
                        )
                        dma_from = qe
    if not nc.is_finalized():
        nc.finalize()
    return nc


def _host_prep(w1_w, w1_b, w2_w):
    bf = ml_dtypes.bfloat16
    # w1augP[m, (d, u, c)] = W1[(c*META+d), m]; row 32 = w1_b
    w1 = w1_w.reshape(C, META, META).transpose(2, 1, 0)      # (m, d, c)
    w1b = w1_b.reshape(C, META).T                            # (d, c)
    w1aug = np.concatenate([w1, w1b[None]], axis=0)          # (33, d, c)
    w1augP = w1aug.reshape(MA, META * C)
    # w2P[e, (j, o)] = w2_w[(o*KK+j), e], e < 32; replicated at 3 bases
    w2 = w2_w.reshape(O, KK, META).transpose(2, 1, 0)        # (e, j, o)
    w2P = w2.reshape(META, KK * O)
    w2quad = np.zeros((96, KK * O), np.float32)
    for i in range(3):
        w2quad[32 * i : 32 * i + 32] = w2P
    identB = np.eye(128, dtype=bf)
    return w1augP.astype(bf), w2quad.astype(bf), identB


def make_core_inputs(meta, x, w1_w, w1_b, w2_w, w2_b):
    """meta (per, 32) f32, x (per, L, C) f32 -> input map for one core."""
    bf = ml_dtypes.bfloat16
    per = meta.shape[0]
    w1augP, w2quad, identB = _host_prep(w1_w, w1_b, w2_w)
    metaT = np.concatenate(
        [meta.T, np.ones((1, per), np.float32)], axis=0
    ).astype(bf)
    # x image: [ (n%2)*64 + c, (n//2)*L + t ]
    xt = np.ascontiguousarray(x.transpose(0, 2, 1)).astype(bf)   # (per, C, L)
    ximg = xt.reshape(per // 2, 2, C, L).transpose(1, 2, 0, 3).reshape(128, (per // 2) * L)
    return {
        "x": np.ascontiguousarray(ximg),
        "metaT": np.ascontiguousarray(metaT),
        "w1augP": w1augP,
        "w2quad": w2quad,
        "identB": identB,
    }


def postprocess_core_output(out_raw, meta, x, w2_b, bl_w=None, bl_b=None):
    """out_raw (LOUT, per*O) bf16 -> (per, LOUT, O) f32 with host bias terms.

    x is the core's (per, L, C) f32 slice (for the w2-bias channel-sum term).
    """
    per = meta.shape[0]
    out = np.asarray(out_raw, dtype=np.float32).reshape(LOUT, per, O).transpose(1, 0, 2)
    # w2 bias term: out[t, o] += sum_j b2[(o,j)] * s[t+j], s = channel sum
    s = x.sum(axis=2)                                        # (per, L)
    b2 = w2_b.reshape(O, KK)                                 # (o, j)
    sw = np.lib.stride_tricks.sliding_window_view(s, KK, axis=1)  # (per, LOUT, KK)
    out = out + sw @ b2.T                                    # (per, LOUT, O)
    if bl_w is not None:
        b = meta @ bl_w.T + bl_b                             # (per, O)
        out = out + b[:, None, :]
    return np.ascontiguousarray(out)


LAST_EXEC_NS = None
_NC_CACHE = {}


def kernel(meta_knowledge, input, w1_w, w1_b, w2_w, w2_b, bl_w, bl_b):
    global LAST_EXEC_NS
    import os

    x_all = np.ascontiguousarray(input.reshape(BN, L, C), dtype=np.float32)

    if PER not in _NC_CACHE:
        _NC_CACHE[PER] = build_program(PER)
    nc = _NC_CACHE[PER]
    in_maps = []
    for i in range(NCORES):
        s = slice(i * PER, (i + 1) * PER)
        in_maps.append(
            make_core_inputs(meta_knowledge[s], x_all[s], w1_w, w1_b, w2_w, w2_b)
        )
    trace = os.environ.get("KM_TRACE", "0") == "1"
    res = run_bass_kernel_spmd(
        nc, in_maps, core_ids=list(range(NCORES)), trace=trace
    )
    if res.exec_time_ns is not None:
        LAST_EXEC_NS = res.exec_time_ns
    outs = []
    for i, r in enumerate(res.results):
        s = slice(i * PER, (i + 1) * PER)
        outs.append(
            postprocess_core_output(
                r["out"], meta_knowledge[s], x_all[s], w2_b, bl_w, bl_b
            )
        )
    out = np.concatenate(outs, axis=0)
    return out.reshape(B, N, LOUT, O)


# revision 5
# speedup vs baseline: 1.0790x; 1.0790x over previous
"""MetaConv1d Trainium2 kernel — v15 (G scheme; pair transposes).

Math (per sample n; device does the two big contractions in bf16/fp32-psum):
  W1c[d, c] = sum_m meta_aug[m, n] * w1aug[m, (d, c)]   (step1, tile-batched;
                                                         w1 bias via meta ones)
  G[e, t']  = sum_c W1c[e, c] * x[c, t']                (matmul1, e in 0..32)
  out[t, o] = sum_{e, j} G[e, t+j] * w2[e, (j, o)]      (matmul2, 3 taps)
Host adds the two cheap bias terms (bl linear; w2-bias x channel-sum conv).

Hardware rule discovered by probing: all matmuls targeting the same PSUM
bank must share the same operand partition base (mixing bases crashes the
device: NRT_EXEC_UNIT error). Hence:
  - G psum tiles are split by sample parity (even samples read x/W1c at
    partition base 0, odd at base 64): each (96, 256) tile takes 6
    same-parity samples at 3 output bases x 2 column halves.
  - out psum tiles are split by G partition base: each (126, 256) tile
    takes 4 samples whose G rows live at the same base 32g.

Cost-model-driven layout (see v4): per-sample matmuls keep N small; psum
evacuations batch 4-6 samples at full partition width; one x DMA and one
out DMA per 120-sample tile; x host-pretransposed to (c, t) bf16.

Sharding: batch*node dim (6624) split evenly over 8 cores (828 each).
"""

import numpy as np
import ml_dtypes

import concourse.mybir as mybir
import concourse.bacc as bacc
from concourse.tile import TileContext
from concourse.bass_utils import run_bass_kernel_spmd

BF16 = mybir.dt.bfloat16
F32 = mybir.dt.float32

B = 32
N = 207
BN = B * N            # 6624
L = 128
C = 64                # in channels
O = 64                # out channels
KK = 3
META = 32
MA = META + 1         # aug (ones row feeds w1 bias in step1)
LOUT = L - KK + 1     # 126
NCORES = 8
PER = BN // NCORES    # 828
NTS = 120             # samples per hypernet tile (multiple of 12)
DZ = 24               # samples per group (2 parities x 12), 12 at the tail
DC = 128              # w1augP inner block: (dup u=2) x (c=64)
TB = 4                # d-blocks per transpose-evac batch


def build_program(per=PER):
    """Per-core Bass program (identical on all 8 cores)."""
    assert per % 12 == 0
    nc = bacc.Bacc("TRN2", target_bir_lowering=False)

    # x image: partition p = (n%2)*64 + c ; col = (n//2)*L + t
    x_d = nc.dram_tensor("x", (128, (per // 2) * L), BF16, kind="ExternalInput")
    metaT_d = nc.dram_tensor("metaT", (MA, per), BF16, kind="ExternalInput")
    # w1augP: [m, d*64 + c] = W1[(c,d), m] (+ w1_b row); the base-64 copy of
    # W1cT is made by a second partition-shifted evac of each transpose
    w1augP_d = nc.dram_tensor("w1augP", (MA, META * C), BF16, kind="ExternalInput")
    # w2quad: rows 32i:32i+32 (i<3) hold w2P[e, j*64+o] (e<32, no bias row)
    w2quad_d = nc.dram_tensor("w2quad", (96, KK * O), BF16, kind="ExternalInput")
    identB_d = nc.dram_tensor("identB", (128, 128), BF16, kind="ExternalInput")
    # out image: [t, n*64 + o] (bf16; host upcasts + adds biases)
    out_d = nc.dram_tensor("out", (LOUT, per * O), BF16, kind="ExternalOutput")

    n_tiles = [(t, min(NTS, per - t)) for t in range(0, per, NTS)]

    with TileContext(nc) as tc:
        with (
            tc.tile_pool(name="const", bufs=1) as cpool,
            tc.tile_pool(name="wpool", bufs=2) as wpool,
            tc.tile_pool(name="xpool", bufs=2) as xpool,
            tc.tile_pool(name="gpool", bufs=3) as gpool,
            tc.tile_pool(name="opool", bufs=2) as opool,
            tc.tile_pool(name="pst", bufs=3, space="PSUM") as pst,
            tc.tile_pool(name="psg", bufs=2, space="PSUM") as psg,
            tc.tile_pool(name="pso", bufs=3, space="PSUM") as pso,
        ):
            w1augP = cpool.tile([MA, META * C], BF16)
            nc.sync.dma_start(w1augP[:, :], w1augP_d[:, :])
            w2quad = cpool.tile([96, KK * O], BF16)
            nc.sync.dma_start(w2quad[:, :], w2quad_d[:, :])
            identB = cpool.tile([128, 128], BF16)
            nc.sync.dma_start(identB[:, :], identB_d[:, :])

            for n0, nt in n_tiles:
                metaT_sb = wpool.tile([MA, nt], BF16, tag="metaT", padded_shape=[MA, NTS])
                nc.gpsimd.dma_start(metaT_sb[:, :], metaT_d[:, n0 : n0 + nt])

                # whole-tile x load: one DMA per 120 samples
                x_sb = xpool.tile(
                    [128, (nt // 2) * L], BF16, tag="xsb",
                    padded_shape=[128, (NTS // 2) * L],
                )
                nc.gpsimd.dma_start(
                    x_sb[:, :], x_d[:, n0 // 2 * L : (n0 + nt) // 2 * L]
                )

                # step1: W1out[n, (d, u, c)] batched over the tile
                W1out = wpool.tile(
                    [nt, META * C], BF16, tag="w1out", padded_shape=[NTS, META * C]
                )
                for k in range(META * C // 512):
                    ps1 = pst.tile([nt, 512], F32, tag="pstile", padded_shape=[NTS, 512])
                    nc.tensor.matmul(
                        ps1[:, :],
                        metaT_sb[:, :],
                        w1augP[:, k * 512 : (k + 1) * 512],
                        start=True,
                        stop=True,
                    )
                    if k % 2:
                        nc.vector.tensor_copy(W1out[:, k * 512 : (k + 1) * 512], ps1[:, :])
                    else:
                        nc.scalar.copy(W1out[:, k * 512 : (k + 1) * 512], ps1[:, :])

                # transpose d-PAIR blocks (nt, 128) -> (128, nt): rows 0:64
                # hold d's W1c rows (c-major), rows 64:128 hold d+1's. Four
                # strided 3D-AP copies per 8-d batch fill both 64-partition
                # halves of W1cT (HW needs the base-64 duplicate).
                W1cT = wpool.tile([128, META * NTS], BF16, tag="w1ct")
                TB8 = 8
                for d0 in range(0, META, TB8):
                    psT = pst.tile(
                        [128, (TB8 // 2) * nt], BF16, tag="pstile",
                        padded_shape=[128, (TB8 // 2) * NTS],
                    )
                    for k in range(TB8 // 2):
                        nc.tensor.transpose(
                            psT[:, k * nt : (k + 1) * nt],
                            W1out[:, (d0 + 2 * k) * C : (d0 + 2 * k + 2) * C],
                            identB[0:nt, 0:nt],
                        )
                    dstv = W1cT[:, :].rearrange("p (e n) -> p e n", e=META)
                    srcv = psT[:, :].rearrange("p (k n) -> p k n", n=nt)
                    for half, eng in ((0, 0), (0, 1), (1, 0), (1, 1)):
                        # src rows half*64: d = d0 + 2k + half; dst eng half
                        dst = dstv[
                            eng * 64 : eng * 64 + 64,
                            d0 + half : d0 + TB8 : 2,
                            0:nt,
                        ]
                        src = srcv[half * 64 : half * 64 + 64]
                        if (half + eng) % 2 == 0:
                            nc.vector.tensor_copy(dst, src)
                        else:
                            nc.scalar.copy(dst, src)
                # lhsT view: [(n%2)*64 + c, e*NTS + ln] with e in 0..32
                W1cT_r = W1cT[:, :].rearrange("p (e n) -> p n e", e=META)

                # per-sample stage in dozens; G tiles split by parity so each
                # psum bank sees a single operand base (HW requirement).
                out_sb = opool.tile(
                    [LOUT, nt * O], BF16, tag="osb", padded_shape=[LOUT, NTS * O]
                )
                dma_from = 0
                out_r = out_sb[:, :].rearrange("t (n z) -> t n z", z=2 * O)
                for q0 in range(0, nt, DZ):
                    gw = min(DZ, nt - q0)      # 24, or 12 at the tail
                    hp = gw // 2               # samples per parity (12 or 6)
                    nb = hp // 3               # G column blocks (4 or 2)
                    Gs = []
                    for par in (0, 1):
                        psG = psg.tile(
                            [96, nb * L], F32, tag="psG", padded_shape=[96, 4 * L]
                        )
                        for i in range(hp):
                            ln = q0 + 2 * i + par
                            nc.tensor.matmul(
                                psG[(i % 3) * 32 : (i % 3) * 32 + 32,
                                    (i // 3) * L : (i // 3) * L + L],
                                W1cT_r[par * 64 : par * 64 + C, ln, :],
                                x_sb[par * 64 : par * 64 + C,
                                     (ln // 2) * L : (ln // 2) * L + L],
                                start=True,
                                stop=True,
                            )
                        Gsb = gpool.tile(
                            [96, nb * L], BF16, tag="gsb", padded_shape=[96, 4 * L]
                        )
                        nc.vector.tensor_copy(Gsb[:, :], psG[:, :])
                        Gs.append(Gsb)

                    # out groups by G partition base 32g: gw/3 samples each,
                    # slot order [e(g), o(g), e(g+3), o(g+3), ...] makes the
                    # out_sb destination a regular strided pattern.
                    for g in range(3):
                        ns_ = gw // 3          # 8 or 4 samples
                        psO = pso.tile(
                            [LOUT, ns_ * O], F32, tag="psO",
                            padded_shape=[LOUT, 8 * O],
                        )
                        k = 0
                        for i in range(g, hp, 3):
                            for par in (0, 1):
                                gcol = (i // 3) * L
                                for j in range(KK):
                                    nc.tensor.matmul(
                                        psO[:, k * O : (k + 1) * O],
                                        Gs[par][32 * g : 32 * g + 32,
                                                gcol + j : gcol + j + LOUT],
                                        w2quad[32 * g : 32 * g + 32,
                                               j * O : (j + 1) * O],
                                        start=(j == 0),
                                        stop=(j == KK - 1),
                                    )
                                k += 1
                        # sample pairs p2 = q0//2 + i for i in {g, g+3, ...}
                        p2 = q0 // 2 + g
                        nc.scalar.copy(
                            out_r[:, p2 : p2 + 3 * (ns_ // 2) - 2 : 3, :],
                            psO[:, :].rearrange("t (a z) -> t a z", z=2 * O),
                        )
                    qe = min(q0 + DZ, nt)
                    last_tile = n0 + nt >= per  # finer chunks at the very end
                    if (2 * qe >= nt and last_tile) or (
                        dma_from == 0 and 2 * qe >= nt
                    ) or qe == nt:
                        nc.gpsimd.dma_start(
                            out_d[:, (n0 + dma_from) * O : (n0 + qe) * O],
                            out_sb[:, dma_from * O : qe * O],
                        )
                        dma_from = qe
    if not nc.is_finalized():
        nc.finalize()
    return nc


def _host_prep(w1_w, w1_b, w2_w):
    bf = ml_dtypes.bfloat16
    # w1augP[m, (d, u, c)] = W1[(c*META+d), m]; row 32 = w1_b
    w1 = w1_w.reshape(C, META, META).transpose(2, 1, 0)      # (m, d, c)
    w1b = w1_b.reshape(C, META).T                            # (d, c)
    w1aug = np.concatenate([w1, w1b[None]], axis=0)          # (33, d, c)
    w1augP = w1aug.reshape(MA, META * C)
    # w2P[e, (j, o)] = w2_w[(o*KK+j), e], e < 32; replicated at 3 bases
    w2 = w2_w.reshape(O, KK, META).transpose(2, 1, 0)        # (e, j, o)
    w2P = w2.reshape(META, KK * O)
    w2quad = np.zeros((96, KK * O), np.float32)
    for i in range(3):
        w2quad[32 * i : 32 * i + 32] = w2P
    identB = np.eye(128, dtype=bf)
    return w1augP.astype(bf), w2quad.astype(bf), identB


def make_core_inputs(meta, x, w1_w, w1_b, w2_w, w2_b):
    """meta (per, 32) f32, x (per, L, C) f32 -> input map for one core."""
    bf = ml_dtypes.bfloat16
    per = meta.shape[0]
    w1augP, w2quad, identB = _host_prep(w1_w, w1_b, w2_w)
    metaT = np.concatenate(
        [meta.T, np.ones((1, per), np.float32)], axis=0
    ).astype(bf)
    # x image: [ (n%2)*64 + c, (n//2)*L + t ]
    xt = np.ascontiguousarray(x.transpose(0, 2, 1)).astype(bf)   # (per, C, L)
    ximg = xt.reshape(per // 2, 2, C, L).transpose(1, 2, 0, 3).reshape(128, (per // 2) * L)
    return {
        "x": np.ascontiguousarray(ximg),
        "metaT": np.ascontiguousarray(metaT),
        "w1augP": w1augP,
        "w2quad": w2quad,
        "identB": identB,
    }


def postprocess_core_output(out_raw, meta, x, w2_b, bl_w=None, bl_b=None):
    """out_raw (LOUT, per*O) bf16 -> (per, LOUT, O) f32 with host bias terms.

    x is the core's (per, L, C) f32 slice (for the w2-bias channel-sum term).
    """
    per = meta.shape[0]
    out = np.asarray(out_raw, dtype=np.float32).reshape(LOUT, per, O).transpose(1, 0, 2)
    # w2 bias term: out[t, o] += sum_j b2[(o,j)] * s[t+j], s = channel sum
    s = x.sum(axis=2)                                        # (per, L)
    b2 = w2_b.reshape(O, KK)                                 # (o, j)
    sw = np.lib.stride_tricks.sliding_window_view(s, KK, axis=1)  # (per, LOUT, KK)
    out = out + sw @ b2.T                                    # (per, LOUT, O)
    if bl_w is not None:
        b = meta @ bl_w.T + bl_b                             # (per, O)
        out = out + b[:, None, :]
    return np.ascontiguousarray(out)


LAST_EXEC_NS = None
_NC_CACHE = {}


def kernel(meta_knowledge, input, w1_w, w1_b, w2_w, w2_b, bl_w, bl_b):
    global LAST_EXEC_NS
    import os

    x_all = np.ascontiguousarray(input.reshape(BN, L, C), dtype=np.float32)

    if PER not in _NC_CACHE:
        _NC_CACHE[PER] = build_program(PER)
    nc = _NC_CACHE[PER]
    in_maps = []
    for i in range(NCORES):
        s = slice(i * PER, (i + 1) * PER)
        in_maps.append(
            make_core_inputs(meta_knowledge[s], x_all[s], w1_w, w1_b, w2_w, w2_b)
        )
    trace = os.environ.get("KM_TRACE", "0") == "1"
    res = run_bass_kernel_spmd(
        nc, in_maps, core_ids=list(range(NCORES)), trace=trace
    )
    if res.exec_time_ns is not None:
        LAST_EXEC_NS = res.exec_time_ns
    outs = []
    for i, r in enumerate(res.results):
        s = slice(i * PER, (i + 1) * PER)
        outs.append(
            postprocess_core_output(
                r["out"], meta_knowledge[s], x_all[s], w2_b, bl_w, bl_b
            )
        )
    out = np.concatenate(outs, axis=0)
    return out.reshape(B, N, LOUT, O)


# revision 6
# speedup vs baseline: 1.0882x; 1.0085x over previous
"""MetaConv1d Trainium2 kernel — v15 (G scheme; pair transposes).

Math (per sample n; device does the two big contractions in bf16/fp32-psum):
  W1c[d, c] = sum_m meta_aug[m, n] * w1aug[m, (d, c)]   (step1, tile-batched;
                                                         w1 bias via meta ones)
  G[e, t']  = sum_c W1c[e, c] * x[c, t']                (matmul1, e in 0..32)
  out[t, o] = sum_{e, j} G[e, t+j] * w2[e, (j, o)]      (matmul2, 3 taps)
Host adds the two cheap bias terms (bl linear; w2-bias x channel-sum conv).

Hardware rule discovered by probing: all matmuls targeting the same PSUM
bank must share the same operand partition base (mixing bases crashes the
device: NRT_EXEC_UNIT error). Hence:
  - G psum tiles are split by sample parity (even samples read x/W1c at
    partition base 0, odd at base 64): each (96, 256) tile takes 6
    same-parity samples at 3 output bases x 2 column halves.
  - out psum tiles are split by G partition base: each (126, 256) tile
    takes 4 samples whose G rows live at the same base 32g.

Cost-model-driven layout (see v4): per-sample matmuls keep N small; psum
evacuations batch 4-6 samples at full partition width; one x DMA and one
out DMA per 120-sample tile; x host-pretransposed to (c, t) bf16.

Sharding: batch*node dim (6624) split evenly over 8 cores (828 each).
"""

import numpy as np
import ml_dtypes

import concourse.mybir as mybir
import concourse.bacc as bacc
from concourse.tile import TileContext
from concourse.bass_utils import run_bass_kernel_spmd

BF16 = mybir.dt.bfloat16
F32 = mybir.dt.float32

B = 32
N = 207
BN = B * N            # 6624
L = 128
C = 64                # in channels
O = 64                # out channels
KK = 3
META = 32
MA = META + 1         # aug (ones row feeds w1 bias in step1)
LOUT = L - KK + 1     # 126
NCORES = 8
PER = BN // NCORES    # 828
NTS = 120             # samples per hypernet tile (multiple of 12)
DZ = 24               # samples per group (2 parities x 12), 12 at the tail
DC = 128              # w1augP inner block: (dup u=2) x (c=64)
TB = 4                # d-blocks per transpose-evac batch


def build_program(per=PER):
    """Per-core Bass program (identical on all 8 cores)."""
    assert per % 12 == 0
    nc = bacc.Bacc("TRN2", target_bir_lowering=False)

    # x image: partition p = (n%2)*64 + c ; col = (n//2)*L + t
    x_d = nc.dram_tensor("x", (128, (per // 2) * L), BF16, kind="ExternalInput")
    metaT_d = nc.dram_tensor("metaT", (MA, per), BF16, kind="ExternalInput")
    # w1augP: [m, d*64 + c] = W1[(c,d), m] (+ w1_b row); the base-64 copy of
    # W1cT is made by a second partition-shifted evac of each transpose
    w1augP_d = nc.dram_tensor("w1augP", (MA, META * C), BF16, kind="ExternalInput")
    # w2quad: rows 32i:32i+32 (i<3) hold w2P[e, j*64+o] (e<32, no bias row)
    w2quad_d = nc.dram_tensor("w2quad", (96, KK * O), BF16, kind="ExternalInput")
    identB_d = nc.dram_tensor("identB", (128, 128), BF16, kind="ExternalInput")
    # out image: [t, n*64 + o] (bf16; host upcasts + adds biases)
    out_d = nc.dram_tensor("out", (LOUT, per * O), BF16, kind="ExternalOutput")

    n_tiles = [(t, min(NTS, per - t)) for t in range(0, per, NTS)]

    with TileContext(nc) as tc:
        with (
            tc.tile_pool(name="const", bufs=1) as cpool,
            tc.tile_pool(name="wpool", bufs=2) as wpool,
            tc.tile_pool(name="xpool", bufs=2) as xpool,
            tc.tile_pool(name="gpool", bufs=3) as gpool,
            tc.tile_pool(name="opool", bufs=2) as opool,
            tc.tile_pool(name="pst", bufs=3, space="PSUM") as pst,
            tc.tile_pool(name="psg", bufs=2, space="PSUM") as psg,
            tc.tile_pool(name="pso", bufs=3, space="PSUM") as pso,
        ):
            w1augP = cpool.tile([MA, META * C], BF16)
            nc.sync.dma_start(w1augP[:, :], w1augP_d[:, :])
            w2quad = cpool.tile([96, KK * O], BF16)
            nc.sync.dma_start(w2quad[:, :], w2quad_d[:, :])
            identB = cpool.tile([128, 128], BF16)
            nc.sync.dma_start(identB[:, :], identB_d[:, :])

            for n0, nt in n_tiles:
                metaT_sb = wpool.tile([MA, nt], BF16, tag="metaT", padded_shape=[MA, NTS])
                nc.gpsimd.dma_start(metaT_sb[:, :], metaT_d[:, n0 : n0 + nt])

                # whole-tile x load: one DMA per 120 samples
                x_sb = xpool.tile(
                    [128, (nt // 2) * L], BF16, tag="xsb",
                    padded_shape=[128, (NTS // 2) * L],
                )
                nc.gpsimd.dma_start(
                    x_sb[:, :], x_d[:, n0 // 2 * L : (n0 + nt) // 2 * L]
                )

                # step1: W1out[n, (d, u, c)] batched over the tile
                W1out = wpool.tile(
                    [nt, META * C], BF16, tag="w1out", padded_shape=[NTS, META * C]
                )
                for k in range(META * C // 512):
                    ps1 = pst.tile([nt, 512], F32, tag="pstile", padded_shape=[NTS, 512])
                    nc.tensor.matmul(
                        ps1[:, :],
                        metaT_sb[:, :],
                        w1augP[:, k * 512 : (k + 1) * 512],
                        start=True,
                        stop=True,
                    )
                    if k % 2:
                        nc.vector.tensor_copy(W1out[:, k * 512 : (k + 1) * 512], ps1[:, :])
                    else:
                        nc.scalar.copy(W1out[:, k * 512 : (k + 1) * 512], ps1[:, :])

                # transpose d-PAIR blocks (nt, 128) -> (128, nt): rows 0:64
                # hold d's W1c rows (c-major), rows 64:128 hold d+1's. Four
                # strided 3D-AP copies per 8-d batch fill both 64-partition
                # halves of W1cT (HW needs the base-64 duplicate).
                W1cT = wpool.tile([128, META * NTS], BF16, tag="w1ct")
                TB8 = 8
                for d0 in range(0, META, TB8):
                    psT = pst.tile(
                        [128, (TB8 // 2) * nt], BF16, tag="pstile",
                        padded_shape=[128, (TB8 // 2) * NTS],
                    )
                    for k in range(TB8 // 2):
                        nc.tensor.transpose(
                            psT[:, k * nt : (k + 1) * nt],
                            W1out[:, (d0 + 2 * k) * C : (d0 + 2 * k + 2) * C],
                            identB[0:nt, 0:nt],
                        )
                    dstv = W1cT[:, :].rearrange("p (e n) -> p e n", e=META)
                    srcv = psT[:, :].rearrange("p (k n) -> p k n", n=nt)
                    for half, eng in ((0, 0), (0, 1), (1, 0), (1, 1)):
                        # src rows half*64: d = d0 + 2k + half; dst eng half
                        dst = dstv[
                            eng * 64 : eng * 64 + 64,
                            d0 + half : d0 + TB8 : 2,
                            0:nt,
                        ]
                        src = srcv[half * 64 : half * 64 + 64]
                        if (half + eng) % 2 == 0:
                            nc.vector.tensor_copy(dst, src)
                        else:
                            nc.scalar.copy(dst, src)
                # lhsT view: [(n%2)*64 + c, e*NTS + ln] with e in 0..32
                W1cT_r = W1cT[:, :].rearrange("p (e n) -> p n e", e=META)

                # per-sample stage in dozens; G tiles split by parity so each
                # psum bank sees a single operand base (HW requirement).
                out_sb = opool.tile(
                    [LOUT, nt * O], BF16, tag="osb", padded_shape=[LOUT, NTS * O]
                )
                dma_from = 0
                out_r = out_sb[:, :].rearrange("t (n z) -> t n z", z=2 * O)
                for q0 in range(0, nt, DZ):
                    gw = min(DZ, nt - q0)      # 24, or 12 at the tail
                    hp = gw // 2               # samples per parity (12 or 6)
                    nb = hp // 3               # G column blocks (4 or 2)
                    Gs = []
                    for par in (0, 1):
                        psG = psg.tile(
                            [96, nb * L], F32, tag="psG", padded_shape=[96, 4 * L]
                        )
                        for i in range(hp):
                            ln = q0 + 2 * i + par
                            nc.tensor.matmul(
                                psG[(i % 3) * 32 : (i % 3) * 32 + 32,
                                    (i // 3) * L : (i // 3) * L + L],
                                W1cT_r[par * 64 : par * 64 + C, ln, :],
                                x_sb[par * 64 : par * 64 + C,
                                     (ln // 2) * L : (ln // 2) * L + L],
                                start=True,
                                stop=True,
                            )
                        Gsb = gpool.tile(
                            [96, nb * L], BF16, tag="gsb", padded_shape=[96, 4 * L]
                        )
                        nc.vector.tensor_copy(Gsb[:, :], psG[:, :])
                        Gs.append(Gsb)

                    # out groups by G partition base 32g: gw/3 samples each,
                    # slot order [e(g), o(g), e(g+3), o(g+3), ...] makes the
                    # out_sb destination a regular strided pattern.
                    for g in range(3):
                        ns_ = gw // 3          # 8 or 4 samples
                        psO = pso.tile(
                            [LOUT, ns_ * O], F32, tag="psO",
                            padded_shape=[LOUT, 8 * O],
                        )
                        k = 0
                        for i in range(g, hp, 3):
                            for par in (0, 1):
                                gcol = (i // 3) * L
                                for j in range(KK):
                                    nc.tensor.matmul(
                                        psO[:, k * O : (k + 1) * O],
                                        Gs[par][32 * g : 32 * g + 32,
                                                gcol + j : gcol + j + LOUT],
                                        w2quad[32 * g : 32 * g + 32,
                                               j * O : (j + 1) * O],
                                        start=(j == 0),
                                        stop=(j == KK - 1),
                                    )
                                k += 1
                        # sample pairs p2 = q0//2 + i for i in {g, g+3, ...}
                        p2 = q0 // 2 + g
                        if g == 1:
                            nc.vector.tensor_copy(
                                out_r[:, p2 : p2 + 3 * (ns_ // 2) - 2 : 3, :],
                                psO[:, :].rearrange("t (a z) -> t a z", z=2 * O),
                            )
                        else:
                            nc.scalar.copy(
                                out_r[:, p2 : p2 + 3 * (ns_ // 2) - 2 : 3, :],
                                psO[:, :].rearrange("t (a z) -> t a z", z=2 * O),
                            )
                    qe = min(q0 + DZ, nt)
                    last_tile = n0 + nt >= per  # finer chunks at the very end
                    if (2 * qe >= nt and last_tile) or (
                        dma_from == 0 and 2 * qe >= nt
                    ) or qe == nt:
                        nc.gpsimd.dma_start(
                            out_d[:, (n0 + dma_from) * O : (n0 + qe) * O],
                            out_sb[:, dma_from * O : qe * O],
                        )
                        dma_from = qe
    if not nc.is_finalized():
        nc.finalize()
    return nc


def _host_prep(w1_w, w1_b, w2_w):
    bf = ml_dtypes.bfloat16
    # w1augP[m, (d, u, c)] = W1[(c*META+d), m]; row 32 = w1_b
    w1 = w1_w.reshape(C, META, META).transpose(2, 1, 0)      # (m, d, c)
    w1b = w1_b.reshape(C, META).T                            # (d, c)
    w1aug = np.concatenate([w1, w1b[None]], axis=0)          # (33, d, c)
    w1augP = w1aug.reshape(MA, META * C)
    # w2P[e, (j, o)] = w2_w[(o*KK+j), e], e < 32; replicated at 3 bases
    w2 = w2_w.reshape(O, KK, META).transpose(2, 1, 0)        # (e, j, o)
    w2P = w2.reshape(META, KK * O)
    w2quad = np.zeros((96, KK * O), np.float32)
    for i in range(3):
        w2quad[32 * i : 32 * i + 32] = w2P
    identB = np.eye(128, dtype=bf)
    return w1augP.astype(bf), w2quad.astype(bf), identB


def make_core_inputs(meta, x, w1_w, w1_b, w2_w, w2_b):
    """meta (per, 32) f32, x (per, L, C) f32 -> input map for one core."""
    bf = ml_dtypes.bfloat16
    per = meta.shape[0]
    w1augP, w2quad, identB = _host_prep(w1_w, w1_b, w2_w)
    metaT = np.concatenate(
        [meta.T, np.ones((1, per), np.float32)], axis=0
    ).astype(bf)
    # x image: [ (n%2)*64 + c, (n//2)*L + t ]
    xt = np.ascontiguousarray(x.transpose(0, 2, 1)).astype(bf)   # (per, C, L)
    ximg = xt.reshape(per // 2, 2, C, L).transpose(1, 2, 0, 3).reshape(128, (per // 2) * L)
    return {
        "x": np.ascontiguousarray(ximg),
        "metaT": np.ascontiguousarray(metaT),
        "w1augP": w1augP,
        "w2quad": w2quad,
        "identB": identB,
    }


def postprocess_core_output(out_raw, meta, x, w2_b, bl_w=None, bl_b=None):
    """out_raw (LOUT, per*O) bf16 -> (per, LOUT, O) f32 with host bias terms.

    x is the core's (per, L, C) f32 slice (for the w2-bias channel-sum term).
    """
    per = meta.shape[0]
    out = np.asarray(out_raw, dtype=np.float32).reshape(LOUT, per, O).transpose(1, 0, 2)
    # w2 bias term: out[t, o] += sum_j b2[(o,j)] * s[t+j], s = channel sum
    s = x.sum(axis=2)                                        # (per, L)
    b2 = w2_b.reshape(O, KK)                                 # (o, j)
    sw = np.lib.stride_tricks.sliding_window_view(s, KK, axis=1)  # (per, LOUT, KK)
    out = out + sw @ b2.T                                    # (per, LOUT, O)
    if bl_w is not None:
        b = meta @ bl_w.T + bl_b                             # (per, O)
        out = out + b[:, None, :]
    return np.ascontiguousarray(out)


LAST_EXEC_NS = None
_NC_CACHE = {}


def kernel(meta_knowledge, input, w1_w, w1_b, w2_w, w2_b, bl_w, bl_b):
    global LAST_EXEC_NS
    import os

    x_all = np.ascontiguousarray(input.reshape(BN, L, C), dtype=np.float32)

    if PER not in _NC_CACHE:
        _NC_CACHE[PER] = build_program(PER)
    nc = _NC_CACHE[PER]
    in_maps = []
    for i in range(NCORES):
        s = slice(i * PER, (i + 1) * PER)
        in_maps.append(
            make_core_inputs(meta_knowledge[s], x_all[s], w1_w, w1_b, w2_w, w2_b)
        )
    trace = os.environ.get("KM_TRACE", "0") == "1"
    res = run_bass_kernel_spmd(
        nc, in_maps, core_ids=list(range(NCORES)), trace=trace
    )
    if res.exec_time_ns is not None:
        LAST_EXEC_NS = res.exec_time_ns
    outs = []
    for i, r in enumerate(res.results):
        s = slice(i * PER, (i + 1) * PER)
        outs.append(
            postprocess_core_output(
                r["out"], meta_knowledge[s], x_all[s], w2_b, bl_w, bl_b
            )
        )
    out = np.concatenate(outs, axis=0)
    return out.reshape(B, N, LOUT, O)


# revision 7
# speedup vs baseline: 1.0948x; 1.0061x over previous
"""MetaConv1d Trainium2 kernel — v15 (G scheme; pair transposes).

Math (per sample n; device does the two big contractions in bf16/fp32-psum):
  W1c[d, c] = sum_m meta_aug[m, n] * w1aug[m, (d, c)]   (step1, tile-batched;
                                                         w1 bias via meta ones)
  G[e, t']  = sum_c W1c[e, c] * x[c, t']                (matmul1, e in 0..32)
  out[t, o] = sum_{e, j} G[e, t+j] * w2[e, (j, o)]      (matmul2, 3 taps)
Host adds the two cheap bias terms (bl linear; w2-bias x channel-sum conv).

Hardware rule discovered by probing: all matmuls targeting the same PSUM
bank must share the same operand partition base (mixing bases crashes the
device: NRT_EXEC_UNIT error). Hence:
  - G psum tiles are split by sample parity (even samples read x/W1c at
    partition base 0, odd at base 64): each (96, 256) tile takes 6
    same-parity samples at 3 output bases x 2 column halves.
  - out psum tiles are split by G partition base: each (126, 256) tile
    takes 4 samples whose G rows live at the same base 32g.

Cost-model-driven layout (see v4): per-sample matmuls keep N small; psum
evacuations batch 4-6 samples at full partition width; one x DMA and one
out DMA per 120-sample tile; x host-pretransposed to (c, t) bf16.

Sharding: batch*node dim (6624) split evenly over 8 cores (828 each).
"""

import numpy as np
import ml_dtypes

import concourse.mybir as mybir
import concourse.bacc as bacc
from concourse.tile import TileContext
from concourse.bass_utils import run_bass_kernel_spmd

BF16 = mybir.dt.bfloat16
F32 = mybir.dt.float32

B = 32
N = 207
BN = B * N            # 6624
L = 128
C = 64                # in channels
O = 64                # out channels
KK = 3
META = 32
MA = META + 1         # aug (ones row feeds w1 bias in step1)
LOUT = L - KK + 1     # 126
NCORES = 8
PER = BN // NCORES    # 828
NTS = 120             # samples per hypernet tile (multiple of 12)
DZ = 24               # samples per group (2 parities x 12), 12 at the tail
DC = 128              # w1augP inner block: (dup u=2) x (c=64)
TB = 4                # d-blocks per transpose-evac batch


def build_program(per=PER):
    """Per-core Bass program (identical on all 8 cores)."""
    assert per % 12 == 0
    nc = bacc.Bacc("TRN2", target_bir_lowering=False)

    # x image: partition p = (n%2)*64 + c ; col = (n//2)*L + t
    x_d = nc.dram_tensor("x", (128, (per // 2) * L), BF16, kind="ExternalInput")
    metaT_d = nc.dram_tensor("metaT", (MA, per), BF16, kind="ExternalInput")
    # w1augP: [m, d*64 + c] = W1[(c,d), m] (+ w1_b row); the base-64 copy of
    # W1cT is made by a second partition-shifted evac of each transpose
    w1augP_d = nc.dram_tensor("w1augP", (MA, META * C), BF16, kind="ExternalInput")
    # w2quad: rows 32i:32i+32 (i<3) hold w2P[e, j*64+o] (e<32, no bias row)
    w2quad_d = nc.dram_tensor("w2quad", (96, KK * O), BF16, kind="ExternalInput")
    identB_d = nc.dram_tensor("identB", (128, 128), BF16, kind="ExternalInput")
    # out image: [t, n*64 + o] (bf16; host upcasts + adds biases)
    out_d = nc.dram_tensor("out", (LOUT, per * O), BF16, kind="ExternalOutput")

    n_tiles = [(t, min(NTS, per - t)) for t in range(0, per, NTS)]

    with TileContext(nc) as tc:
        with (
            tc.tile_pool(name="const", bufs=1) as cpool,
            tc.tile_pool(name="wpool", bufs=2) as wpool,
            tc.tile_pool(name="xpool", bufs=2) as xpool,
            tc.tile_pool(name="gpool", bufs=3) as gpool,
            tc.tile_pool(name="opool", bufs=2) as opool,
            tc.tile_pool(name="pst", bufs=3, space="PSUM") as pst,
            tc.tile_pool(name="psg", bufs=2, space="PSUM") as psg,
            tc.tile_pool(name="pso", bufs=3, space="PSUM") as pso,
        ):
            w1augP = cpool.tile([MA, META * C], BF16)
            nc.sync.dma_start(w1augP[:, :], w1augP_d[:, :])
            w2quad = cpool.tile([96, KK * O], BF16)
            nc.sync.dma_start(w2quad[:, :], w2quad_d[:, :])
            identB = cpool.tile([128, 128], BF16)
            nc.sync.dma_start(identB[:, :], identB_d[:, :])

            for n0, nt in n_tiles:
                metaT_sb = wpool.tile([MA, nt], BF16, tag="metaT", padded_shape=[MA, NTS])
                nc.gpsimd.dma_start(metaT_sb[:, :], metaT_d[:, n0 : n0 + nt])

                # whole-tile x load: one DMA per 120 samples
                x_sb = xpool.tile(
                    [128, (nt // 2) * L], BF16, tag="xsb",
                    padded_shape=[128, (NTS // 2) * L],
                )
                nc.gpsimd.dma_start(
                    x_sb[:, :], x_d[:, n0 // 2 * L : (n0 + nt) // 2 * L]
                )

                # step1: W1out[n, (d, u, c)] batched over the tile
                W1out = wpool.tile(
                    [nt, META * C], BF16, tag="w1out", padded_shape=[NTS, META * C]
                )
                for k in range(META * C // 512):
                    ps1 = pst.tile([nt, 512], F32, tag="pstile", padded_shape=[NTS, 512])
                    nc.tensor.matmul(
                        ps1[:, :],
                        metaT_sb[:, :],
                        w1augP[:, k * 512 : (k + 1) * 512],
                        start=True,
                        stop=True,
                    )
                    if k % 2:
                        nc.vector.tensor_copy(W1out[:, k * 512 : (k + 1) * 512], ps1[:, :])
                    else:
                        nc.scalar.copy(W1out[:, k * 512 : (k + 1) * 512], ps1[:, :])

                # transpose d-PAIR blocks (nt, 128) -> (128, nt): rows 0:64
                # hold d's W1c rows (c-major), rows 64:128 hold d+1's. Four
                # strided 3D-AP copies per 8-d batch fill both 64-partition
                # halves of W1cT (HW needs the base-64 duplicate).
                W1cT = wpool.tile([128, META * NTS], BF16, tag="w1ct")
                TB8 = 8
                for d0 in range(0, META, TB8):
                    psT = pst.tile(
                        [128, (TB8 // 2) * nt], BF16, tag="pstile",
                        padded_shape=[128, (TB8 // 2) * NTS],
                    )
                    for k in range(TB8 // 2):
                        nc.tensor.transpose(
                            psT[:, k * nt : (k + 1) * nt],
                            W1out[:, (d0 + 2 * k) * C : (d0 + 2 * k + 2) * C],
                            identB[0:nt, 0:nt],
                        )
                    dstv = W1cT[:, :].rearrange("p (e n) -> p e n", e=META)
                    srcv = psT[:, :].rearrange("p (k n) -> p k n", n=nt)
                    for half, eng in ((0, 0), (0, 1), (1, 0), (1, 1)):
                        # src rows half*64: d = d0 + 2k + half; dst eng half
                        dst = dstv[
                            eng * 64 : eng * 64 + 64,
                            d0 + half : d0 + TB8 : 2,
                            0:nt,
                        ]
                        src = srcv[half * 64 : half * 64 + 64]
                        if (half + eng) % 2 == 0 or (n0 == 0 and eng == 0):
                            nc.vector.tensor_copy(dst, src)
                        else:
                            nc.scalar.copy(dst, src)
                # lhsT view: [(n%2)*64 + c, e*NTS + ln] with e in 0..32
                W1cT_r = W1cT[:, :].rearrange("p (e n) -> p n e", e=META)

                # per-sample stage in dozens; G tiles split by parity so each
                # psum bank sees a single operand base (HW requirement).
                out_sb = opool.tile(
                    [LOUT, nt * O], BF16, tag="osb", padded_shape=[LOUT, NTS * O]
                )
                dma_from = 0
                out_r = out_sb[:, :].rearrange("t (n z) -> t n z", z=2 * O)
                for q0 in range(0, nt, DZ):
                    gw = min(DZ, nt - q0)      # 24, or 12 at the tail
                    hp = gw // 2               # samples per parity (12 or 6)
                    nb = hp // 3               # G column blocks (4 or 2)
                    Gs = []
                    for par in (0, 1):
                        psG = psg.tile(
                            [96, nb * L], F32, tag="psG", padded_shape=[96, 4 * L]
                        )
                        for i in range(hp):
                            ln = q0 + 2 * i + par
                            nc.tensor.matmul(
                                psG[(i % 3) * 32 : (i % 3) * 32 + 32,
                                    (i // 3) * L : (i // 3) * L + L],
                                W1cT_r[par * 64 : par * 64 + C, ln, :],
                                x_sb[par * 64 : par * 64 + C,
                                     (ln // 2) * L : (ln // 2) * L + L],
                                start=True,
                                stop=True,
                            )
                        Gsb = gpool.tile(
                            [96, nb * L], BF16, tag="gsb", padded_shape=[96, 4 * L]
                        )
                        nc.vector.tensor_copy(Gsb[:, :], psG[:, :])
                        Gs.append(Gsb)

                    # out groups by G partition base 32g: gw/3 samples each,
                    # slot order [e(g), o(g), e(g+3), o(g+3), ...] makes the
                    # out_sb destination a regular strided pattern.
                    for g in range(3):
                        ns_ = gw // 3          # 8 or 4 samples
                        psO = pso.tile(
                            [LOUT, ns_ * O], F32, tag="psO",
                            padded_shape=[LOUT, 8 * O],
                        )
                        k = 0
                        for i in range(g, hp, 3):
                            for par in (0, 1):
                                gcol = (i // 3) * L
                                for j in range(KK):
                                    nc.tensor.matmul(
                                        psO[:, k * O : (k + 1) * O],
                                        Gs[par][32 * g : 32 * g + 32,
                                                gcol + j : gcol + j + LOUT],
                                        w2quad[32 * g : 32 * g + 32,
                                               j * O : (j + 1) * O],
                                        start=(j == 0),
                                        stop=(j == KK - 1),
                                    )
                                k += 1
                        # sample pairs p2 = q0//2 + i for i in {g, g+3, ...}
                        p2 = q0 // 2 + g
                        if g == 1:
                            nc.vector.tensor_copy(
                                out_r[:, p2 : p2 + 3 * (ns_ // 2) - 2 : 3, :],
                                psO[:, :].rearrange("t (a z) -> t a z", z=2 * O),
                            )
                        else:
                            nc.scalar.copy(
                                out_r[:, p2 : p2 + 3 * (ns_ // 2) - 2 : 3, :],
                                psO[:, :].rearrange("t (a z) -> t a z", z=2 * O),
                            )
                    qe = min(q0 + DZ, nt)
                    last_tile = n0 + nt >= per  # finer chunks at the very end
                    if (2 * qe >= nt and last_tile) or (
                        dma_from == 0 and 2 * qe >= nt
                    ) or qe == nt:
                        nc.gpsimd.dma_start(
                            out_d[:, (n0 + dma_from) * O : (n0 + qe) * O],
                            out_sb[:, dma_from * O : qe * O],
                        )
                        dma_from = qe
    if not nc.is_finalized():
        nc.finalize()
    return nc


def _host_prep(w1_w, w1_b, w2_w):
    bf = ml_dtypes.bfloat16
    # w1augP[m, (d, u, c)] = W1[(c*META+d), m]; row 32 = w1_b
    w1 = w1_w.reshape(C, META, META).transpose(2, 1, 0)      # (m, d, c)
    w1b = w1_b.reshape(C, META).T                            # (d, c)
    w1aug = np.concatenate([w1, w1b[None]], axis=0)          # (33, d, c)
    w1augP = w1aug.reshape(MA, META * C)
    # w2P[e, (j, o)] = w2_w[(o*KK+j), e], e < 32; replicated at 3 bases
    w2 = w2_w.reshape(O, KK, META).transpose(2, 1, 0)        # (e, j, o)
    w2P = w2.reshape(META, KK * O)
    w2quad = np.zeros((96, KK * O), np.float32)
    for i in range(3):
        w2quad[32 * i : 32 * i + 32] = w2P
    identB = np.eye(128, dtype=bf)
    return w1augP.astype(bf), w2quad.astype(bf), identB


def make_core_inputs(meta, x, w1_w, w1_b, w2_w, w2_b):
    """meta (per, 32) f32, x (per, L, C) f32 -> input map for one core."""
    bf = ml_dtypes.bfloat16
    per = meta.shape[0]
    w1augP, w2quad, identB = _host_prep(w1_w, w1_b, w2_w)
    metaT = np.concatenate(
        [meta.T, np.ones((1, per), np.float32)], axis=0
    ).astype(bf)
    # x image: [ (n%2)*64 + c, (n//2)*L + t ]
    xt = np.ascontiguousarray(x.transpose(0, 2, 1)).astype(bf)   # (per, C, L)
    ximg = xt.reshape(per // 2, 2, C, L).transpose(1, 2, 0, 3).reshape(128, (per // 2) * L)
    return {
        "x": np.ascontiguousarray(ximg),
        "metaT": np.ascontiguousarray(metaT),
        "w1augP": w1augP,
        "w2quad": w2quad,
        "identB": identB,
    }


def postprocess_core_output(out_raw, meta, x, w2_b, bl_w=None, bl_b=None):
    """out_raw (LOUT, per*O) bf16 -> (per, LOUT, O) f32 with host bias terms.

    x is the core's (per, L, C) f32 slice (for the w2-bias channel-sum term).
    """
    per = meta.shape[0]
    out = np.asarray(out_raw, dtype=np.float32).reshape(LOUT, per, O).transpose(1, 0, 2)
    # w2 bias term: out[t, o] += sum_j b2[(o,j)] * s[t+j], s = channel sum
    s = x.sum(axis=2)                                        # (per, L)
    b2 = w2_b.reshape(O, KK)                                 # (o, j)
    sw = np.lib.stride_tricks.sliding_window_view(s, KK, axis=1)  # (per, LOUT, KK)
    out = out + sw @ b2.T                                    # (per, LOUT, O)
    if bl_w is not None:
        b = meta @ bl_w.T + bl_b                             # (per, O)
        out = out + b[:, None, :]
    return np.ascontiguousarray(out)


LAST_EXEC_NS = None
_NC_CACHE = {}


def kernel(meta_knowledge, input, w1_w, w1_b, w2_w, w2_b, bl_w, bl_b):
    global LAST_EXEC_NS
    import os

    x_all = np.ascontiguousarray(input.reshape(BN, L, C), dtype=np.float32)

    if PER not in _NC_CACHE:
        _NC_CACHE[PER] = build_program(PER)
    nc = _NC_CACHE[PER]
    in_maps = []
    for i in range(NCORES):
        s = slice(i * PER, (i + 1) * PER)
        in_maps.append(
            make_core_inputs(meta_knowledge[s], x_all[s], w1_w, w1_b, w2_w, w2_b)
        )
    trace = os.environ.get("KM_TRACE", "0") == "1"
    res = run_bass_kernel_spmd(
        nc, in_maps, core_ids=list(range(NCORES)), trace=trace
    )
    if res.exec_time_ns is not None:
        LAST_EXEC_NS = res.exec_time_ns
    outs = []
    for i, r in enumerate(res.results):
        s = slice(i * PER, (i + 1) * PER)
        outs.append(
            postprocess_core_output(
                r["out"], meta_knowledge[s], x_all[s], w2_b, bl_w, bl_b
            )
        )
    out = np.concatenate(outs, axis=0)
    return out.reshape(B, N, LOUT, O)
